# revision 2
# baseline (speedup 1.0000x reference)
"""Trainium2 Bass kernel: fused AdaRMSNorm + QK-RMSNorm/RoPE attention.

Sharding: 8 cores = 2 batch x 4 head-groups (8 heads each).
vs v2: per-head RMS stats via gpsimd partition_all_reduce (no PE stat
matmuls, no PSUM pressure), attention interleaved with the q/k projection
per head-group, v computed first, shared PSUM tag for projection/score
matmuls, direct PSUM->DRAM output DMA.
"""

import numpy as np
from ml_dtypes import bfloat16

B, L, D, HD, DC = 2, 2048, 2048, 64, 2048
NH = D // HD
EPS = float(np.finfo(np.float32).eps)
NCORES = 8
JL = 512
NHL = 8


# ---------------------------------------------------------------- host prep
def _host_prep(x, condition, rope, w_ada, w_qkv, w_out, qk_w):
    hd_idx = np.arange(HD)
    sign = np.where(hd_idx % 2 == 0, -1.0, 1.0).astype(np.float32)
    partner = hd_idx ^ 1
    rope0, rope1 = rope[0].T, rope[1].T                      # [64, L]
    ropeA = (rope0 * qk_w[:, None]).astype(np.float32)
    ropeB = (rope1 * (sign * qk_w[partner])[:, None]).astype(np.float32)
    scl = np.float32(1.0 / np.sqrt(HD))
    ropeA2 = np.tile(ropeA, (2, 1))
    ropeB2 = np.tile(ropeB, (2, 1))
    bf = lambda a: np.ascontiguousarray(a.astype(bfloat16))
    ra_t = bf(ropeA2)
    rb_t = bf(ropeB2)

    in_maps = []
    for dev in range(NCORES):
        b, g = dev // 4, dev % 4
        ss = (w_ada @ condition[b]).astype(np.float32)
        shift, s1 = ss[:D], (1.0 + ss[D:]).astype(np.float32)
        Wq = w_qkv[g * JL:(g + 1) * JL]
        Wk = w_qkv[D + g * JL:D + (g + 1) * JL]
        Wv = w_qkv[2 * D + g * JL:2 * D + (g + 1) * JL]
        m = {
            "xT": bf(x[b].T),
            "wq": bf((Wq * s1[None, :]).T),
            "wk": bf((Wk * s1[None, :]).T),
            "wv": bf((Wv * s1[None, :]).T),
            "cqr": np.ascontiguousarray(
                (Wq @ shift).astype(bfloat16).reshape(1, 4, 128)),
            "ckr": np.ascontiguousarray(
                (Wk @ shift).astype(bfloat16).reshape(1, 4, 128)),
            "cqt": np.ascontiguousarray(
                (Wq @ shift).astype(np.float32).reshape(4, 128).T),
            "ckt": np.ascontiguousarray(
                (Wk @ shift).astype(np.float32).reshape(4, 128).T),
            "cv": (Wv @ shift).astype(bfloat16).reshape(1, JL),
            "ra": ra_t, "rb": rb_t,
            "woT": bf(w_out[:, g * JL:(g + 1) * JL].T),      # [512 j, 2048 d]
        }
        in_maps.append(m)
    return in_maps


# ---------------------------------------------------------------- bass build
def _build_nc():
    import concourse.bass as bass
    import concourse.mybir as mybir
    import concourse.tile as tile
    from concourse import bacc
    from concourse import bass_isa
    from concourse import library_config

    f32 = mybir.dt.float32
    f32r = mybir.dt.float32r
    bf16 = mybir.dt.bfloat16
    AF = mybir.ActivationFunctionType
    MUL = mybir.AluOpType.mult
    ADD = mybir.AluOpType.add

    nc = bacc.Bacc("TRN2", target_bir_lowering=False, debug=False, num_devices=8)
    xT_d = nc.dram_tensor("xT", [D, L], bf16, kind="ExternalInput")
    w_d = {nm: nc.dram_tensor(nm, [D, JL], bf16, kind="ExternalInput")
           for nm in ("wq", "wk", "wv")}
    cqt_d = nc.dram_tensor("cqr", [1, 4, 128], bf16, kind="ExternalInput")
    ckt_d = nc.dram_tensor("ckr", [1, 4, 128], bf16, kind="ExternalInput")
    cqs_d = nc.dram_tensor("cqt", [128, 4], f32, kind="ExternalInput")
    cks_d = nc.dram_tensor("ckt", [128, 4], f32, kind="ExternalInput")
    cv_d = nc.dram_tensor("cv", [1, JL], bf16, kind="ExternalInput")
    rope_d = {nm: nc.dram_tensor(nm, [128, L], bf16, kind="ExternalInput")
              for nm in ("ra", "rb")}
    woT_d = nc.dram_tensor("woT", [JL, D], bf16, kind="ExternalInput")
    out_d = nc.dram_tensor("out", [D, L], bf16, kind="ExternalOutput")

    def r(ap):
        return ap.bitcast(f32r)

    shuf = [i ^ 1 for i in range(32)]
    RADD = bass_isa.ReduceOp.add

    with tile.TileContext(nc) as tc, \
            nc.allow_low_precision(reason="bf16 kernel"):
        with (
            tc.tile_pool(name="consts", bufs=1) as consts,
            tc.tile_pool(name="qkT", bufs=1) as qkTp,
            tc.tile_pool(name="vsb", bufs=1) as vsbp,
            tc.tile_pool(name="dram", bufs=1, space="DRAM") as dpool,
        ):
            # preload the ln+exp activation table once; the fixpoint pass
            # then finds it resident on every path and inserts no reloads
            from concourse.hw_specs import get_activation_tables
            _tabs = list(get_activation_tables(nc.m.arch).keys())
            nc.scalar.add_instruction(mybir.InstLoadActFuncSet(
                name=nc.get_next_instruction_name(), ins=[], outs=[],
                act_func_set_id=_tabs.index("natural_log_exp_and_others")))
            ones_b = consts.tile([128, 1], bf16)
            nc.vector.memset(ones_b, 1.0)
            s2 = consts.tile([128, 2], bf16)
            nc.vector.memset(s2, 0.0)
            nc.vector.memset(s2[0:64, 0:1], 1.0)
            nc.vector.memset(s2[64:128, 1:2], 1.0)
            eps1 = consts.tile([1, 1], f32)
            nc.vector.memset(eps1, EPS)
            eps65 = consts.tile([65, 1], f32)
            nc.vector.memset(eps65, EPS)
            cq_t = consts.tile([1, 4, 128], bf16)
            nc.sync.dma_start(out=cq_t, in_=cqt_d[:, :, :])
            ck_t = consts.tile([1, 4, 128], bf16)
            nc.sync.dma_start(out=ck_t, in_=ckt_d[:, :, :])
            cq_s = consts.tile([128, 4], f32)
            nc.sync.dma_start(out=cq_s, in_=cqs_d[:, :])
            ck_s = consts.tile([128, 4], f32)
            nc.sync.dma_start(out=ck_s, in_=cks_d[:, :])
            cv_t = consts.tile([1, JL], bf16)
            nc.sync.dma_start(out=cv_t, in_=cv_d[:, :])
            cvb = consts.tile([128, JL], bf16)
            rr_row = consts.tile([1, L], bf16)
            rrb_sb = consts.tile([128, L], bf16)
            rr_col = consts.tile([128, 16], bf16)
            ones_row_b = consts.tile([1, 128], bf16)
            nc.vector.memset(ones_row_b, 1.0)

            # persistent bf16 tiles: q/k (roped+normalized) and v (+ones col)
            qkT = {"q": [], "k": []}
            for nm in ("q", "k"):
                for m in range(4):
                    qkT[nm].append(qkTp.tile([128, L], bf16, tag=f"{nm}T{m}",
                                             name=f"{nm}T{m}"))
            vsb = []
            for i in range(16):
                t = vsbp.tile([128, NHL, HD + 1], bf16, tag=f"v{i}", name=f"v{i}")
                nc.vector.memset(t[:, :, HD:HD + 1], 1.0)
                vsb.append(t)
            # oT tiles: oT[0] fresh, oT[g>=1] reuses the dead q-tile of group g-1
            oT = [qkTp.tile([128, L], bf16, tag="oT0", name="oT0")]

            with (
                tc.tile_pool(name="xs", bufs=1) as xsp,
                tc.tile_pool(name="wq", bufs=1) as wqp,
                tc.tile_pool(name="wvk", bufs=1) as wvkp,
                tc.tile_pool(name="ropes", bufs=1) as rp,
                tc.tile_pool(name="tmps", bufs=2) as tmp,
                tc.tile_pool(name="etp", bufs=1) as etp,
            ):
                # ---- A: stream x (+v weights), square, col sums-of-squares --
                xs, wtv = [], []
                with tc.tile_pool(name="psA1", bufs=1, space="PSUM") as psA1:
                    cvb_ps = psA1.tile([128, JL], f32, tag="cvb")
                    nc.tensor.matmul(cvb_ps, lhsT=ones_row_b, rhs=cv_t,
                                     start=True, stop=True)
                    nc.scalar.copy(cvb, cvb_ps)
                    ssq = psA1.tile([1, L], f32, tag="ssq")
                    for i in range(16):
                        t = xsp.tile([128, L], bf16, tag=f"xs{i}", name=f"xs{i}")
                        nc.sync.dma_start(out=t, in_=xT_d[i * 128:(i + 1) * 128, :])
                        xs.append(t)
                        tv = wvkp.tile([128, JL], bf16, tag=f"wk{i}", name=f"wv{i}")
                        nc.sync.dma_start(out=tv, in_=w_d["wv"][i * 128:(i + 1) * 128, :])
                        wtv.append(tv)
                        for hh in range(2):
                            sqa = tmp.tile([128, 1024], bf16, tag="sq1",
                                           name="sqA", bufs=2)
                            sl = slice(hh * 1024, (hh + 1) * 1024)
                            nc.vector.tensor_mul(sqa, t[:, sl], t[:, sl])
                            for c in range(2):
                                cg = hh * 2 + c
                                nc.tensor.matmul(ssq[:, cg * JL:(cg + 1) * JL],
                                                 lhsT=ones_b[:, 0:1],
                                                 rhs=sqa[:, c * JL:(c + 1) * JL],
                                                 start=(i == 0), stop=(i == 15))
                    nc.scalar.activation(ssq, ssq, AF.Ln, bias=eps1,
                                         scale=1.0 / D)
                    # rr_row = exp(0.5*ln(mean x^2 + eps)) = 1/rr; Ln+Exp share
                    # one activation table with the attention exps (no reloads)
                    nc.scalar.activation(rr_row, ssq, AF.Exp, scale=0.5)

                # ---- rr broadcasts + v projection ----
                with tc.tile_pool(name="psA2", bufs=1, space="PSUM") as psA2:
                    rr_scr = dpool.tile([16, 128], bf16, tag="rrscr")
                    nc.sync.dma_start(out=rr_scr,
                                      in_=rr_row.rearrange("a (m p) -> a m p", p=128))
                    inv_col = consts.tile([128, 16], bf16)
                    nc.sync.dma_start(out=inv_col,
                                      in_=rr_scr.rearrange("m p -> p m"))
                    nc.vector.reciprocal(rr_col, inv_col)
                    # prefetch q/k weights + rope tables (DMA slack here)
                    wtq = []
                    for k16 in range(16):
                        t = wqp.tile([128, JL], bf16, tag=f"wq{k16}",
                                     name=f"wq{k16}")
                        nc.sync.dma_start(
                            out=t, in_=w_d["wq"][k16 * 128:(k16 + 1) * 128, :])
                        wtq.append(t)
                    rope_t = {}
                    for nm in ("ra", "rb"):
                        rope_t[nm] = rp.tile([128, L], bf16, tag=nm, name=nm)
                        nc.sync.dma_start(out=rope_t[nm], in_=rope_d[nm][:, :])
                    cvb_r = cvb.rearrange("p (h d) -> p h d", h=NHL)
                    for mL in range(16):
                        pv = psA2.tile([128, JL], f32, tag="pv", bufs=3)
                        for k16 in range(16):
                            nc.tensor.matmul(
                                pv, lhsT=xs[k16][:, mL * 128:(mL + 1) * 128],
                                rhs=wtv[k16], start=(k16 == 0), stop=(k16 == 15))
                        nc.vector.scalar_tensor_tensor(
                            out=vsb[mL][:, :, 0:HD],
                            in0=pv.rearrange("p (h d) -> p h d", h=NHL),
                            scalar=rr_col[:, mL:mL + 1],
                            in1=cvb_r, op0=MUL, op1=ADD)
                    # rr broadcast for the q/k scale (needed from the first
                    # projection's m1 on; emitted after v so it cannot stall
                    # the PE stream on the adaRMS stats chain)
                    for c in range(4):
                        rrb_ps = psA2.tile([128, JL], f32, tag="pv", bufs=3)
                        nc.tensor.matmul(rrb_ps, lhsT=ones_row_b,
                                         rhs=rr_row[:, c * JL:(c + 1) * JL],
                                         start=True, stop=True)
                        nc.vector.reciprocal(rrb_sb[:, c * JL:(c + 1) * JL],
                                             rrb_ps)
                    # k weights into the wv slots (v matmuls hold them until done)
                    wtk = []
                    for k16 in range(16):
                        t = wvkp.tile([128, JL], bf16, tag=f"wk{k16}",
                                      name=f"wk{k16}")
                        nc.sync.dma_start(
                            out=t, in_=w_d["wk"][k16 * 128:(k16 + 1) * 128, :])
                        wtk.append(t)

                # ---- interleaved: per head-group, q/k proj+norm+rope and
                # attention, with projection matmuls spread through the
                # attention chunk stream so the scalar exp pipe never bunches.
                with tc.tile_pool(name="psCE", bufs=1, space="PSUM") as psCE:
                    pending = []  # deferred non-PE tails of prior units

                    def flush_pending():
                        while pending:
                            pending.pop(0)()

                    def qk_start(nm, m, half):
                        ps = psCE.tile([128, 1024], f32, tag="mm", bufs=1)
                        return {"nm": nm, "m": m, "half": half, "ps": ps,
                                "k16": 0, "mm_bias": m != 0}

                    def qk_mms(u, n):
                        ps, m, half = u["ps"], u["m"], u["half"]
                        wt = wtq if u["nm"] == "q" else wtk
                        for _ in range(n):
                            k16 = u["k16"]
                            if k16 >= 16:
                                return
                            u["k16"] += 1
                            for n2 in range(2):
                                nc.tensor.matmul(
                                    ps[:, n2 * JL:(n2 + 1) * JL],
                                    lhsT=wt[k16][:, m * 128:(m + 1) * 128],
                                    rhs=xs[k16][:, half * 1024 + n2 * JL:
                                                half * 1024 + (n2 + 1) * JL],
                                    start=(k16 == 0), stop=False)

                    def qk_finish(u):
                        nm, m, half, ps = u["nm"], u["m"], u["half"], u["ps"]
                        ra = rope_t["ra"]
                        rb = rope_t["rb"]
                        sl = slice(half * 1024, (half + 1) * 1024)
                        if u["mm_bias"]:
                            qk_mms(u, 16 - u["k16"])
                            c_t = cq_t if nm == "q" else ck_t
                            for n2 in range(2):
                                # bias row: + c[j]*(1/rr)[l]; the rr scale in
                                # m1 then yields rr*(Wx) + c exactly
                                nc.tensor.matmul(
                                    ps[:, n2 * JL:(n2 + 1) * JL],
                                    lhsT=c_t[0:1, m, :],
                                    rhs=rr_row[:, half * 1024 + n2 * JL:
                                               half * 1024 + (n2 + 1) * JL],
                                    start=False, stop=True)
                        else:
                            # group 0: close the accumulation on the last k16
                            # (no PE dependency on the adaRMS stats) and add
                            # the bias on the scalar engine instead
                            qk_mms(u, 15 - u["k16"])
                            wt = wtq if nm == "q" else wtk
                            for n2 in range(2):
                                nc.tensor.matmul(
                                    ps[:, n2 * JL:(n2 + 1) * JL],
                                    lhsT=wt[15][:, m * 128:(m + 1) * 128],
                                    rhs=xs[15][:, half * 1024 + n2 * JL:
                                               half * 1024 + (n2 + 1) * JL],
                                    start=False, stop=True)
                            u["k16"] = 16
                        m1 = tmp.tile([128, 1024], bf16, tag="m1", name="m1",
                                      bufs=2)
                        nc.vector.tensor_mul(m1, ps, rrb_sb[:, sl])
                        if not u["mm_bias"]:
                            c_s = cq_s if nm == "q" else ck_s
                            nc.scalar.activation(m1, m1, AF.Identity,
                                                 bias=c_s[:, m:m + 1])
                        sq = tmp.tile([128, 1024], bf16, tag="sq1", name="sq1",
                                      bufs=2)
                        nc.vector.tensor_mul(sq, m1, m1)
                        qs = tmp.tile([128, 1024], bf16, tag="qs", name="qs",
                                      bufs=2)
                        nc.vector.stream_shuffle(qs, m1, shuf)
                        # per-head sum of squares into PSUM rows 0 / 64
                        # (base partitions 0 and 64 are the HW-legal offsets)
                        ph = psCE.tile([128, 1024], f32, tag="pp", bufs=2)
                        for lc in range(2):
                            cs = slice(lc * JL, (lc + 1) * JL)
                            nc.tensor.matmul(ph[0:1, cs], lhsT=s2[:, 0:1],
                                             rhs=sq[:, cs], start=True, stop=True)
                            nc.tensor.matmul(ph[64:65, cs], lhsT=s2[:, 1:2],
                                             rhs=sq[:, cs], start=True, stop=True)
                        st = tmp.tile([128, 1024], bf16, tag="st", name="st",
                                      bufs=2)
                        nc.scalar.activation(ph[0:65, :], ph[0:65, :], AF.Ln,
                                             bias=eps65, scale=1.0 / HD)
                        nc.scalar.activation(st[0:65, :], ph[0:65, :], AF.Exp,
                                             scale=-0.5)
                        stb = tmp.tile([128, 1024], bf16, tag="stb",
                                       name="stb", bufs=2)
                        nc.sync.dma_start(
                            out=stb[0:64, :],
                            in_=st[0:1, :].unsqueeze(1).to_broadcast([1, 64, 1024]))
                        nc.sync.dma_start(
                            out=stb[64:128, :],
                            in_=st[64:65, :].unsqueeze(1).to_broadcast([1, 64, 1024]))

                        def tail():
                            nc.vector.tensor_mul(m1, m1, stb)        # qn
                            nc.vector.tensor_mul(qs, qs, stb)        # qn-shuf
                            nc.vector.tensor_mul(m1, m1, ra[:, sl])  # t1
                            nc.vector.tensor_mul(qs, qs, rb[:, sl])  # t2
                            nc.vector.tensor_add(qkT[nm][m][:, sl], m1, qs)

                        pending.append(tail)

                    def e_unit(h, lhf, oTt, qk=None):
                        qr_h = qkT["q"][h // 2][(h % 2) * 64:(h % 2) * 64 + 64, :]
                        kr_h = qkT["k"][h // 2][(h % 2) * 64:(h % 2) * 64 + 64, :]
                        ets = []

                        def s_chunk(l2c):
                            pS = psCE.tile([128, 1024], f32, tag="pp", bufs=2)
                            for n2 in range(2):
                                nc.tensor.matmul(
                                    pS[:, n2 * JL:(n2 + 1) * JL],
                                    lhsT=kr_h[:, l2c * 128:(l2c + 1) * 128],
                                    rhs=qr_h[:, lhf * 1024 + n2 * JL:
                                             lhf * 1024 + (n2 + 1) * JL],
                                    start=True, stop=True)
                            et = etp.tile([128, 1024], bf16, tag="et",
                                          name="et", bufs=3)
                            nc.scalar.activation(et, pS, AF.Exp,
                                                 scale=float(1.0 / np.sqrt(HD)))
                            ets.append(et)

                        s_chunk(0)
                        s_chunk(1)
                        flush_pending()
                        po = psCE.tile([HD + 1, 1024], f32, tag="po", bufs=1)

                        def po_chunk(l2c):
                            et = ets[l2c]
                            for n2 in range(2):
                                nc.tensor.matmul(
                                    po[:, n2 * JL:(n2 + 1) * JL],
                                    lhsT=vsb[l2c][:, h, :],
                                    rhs=et[:, n2 * JL:(n2 + 1) * JL],
                                    start=(l2c == 0), stop=(l2c == 15))

                        for l2c in range(2, 16):
                            s_chunk(l2c)
                            po_chunk(l2c - 2)
                            if qk is not None:
                                qk_mms(qk, 1)
                        po_chunk(14)
                        po_chunk(15)
                        if qk is not None:
                            qk_finish(qk)

                        def tail():
                            rd = tmp.tile([1, 1024], bf16, tag="rd", name="rd",
                                          bufs=1)
                            nc.vector.reciprocal(rd, po[HD:HD + 1, :])
                            rdb = tmp.tile([64, 1024], bf16, tag="rdb",
                                           name="rdb", bufs=1)
                            nc.sync.dma_start(
                                out=rdb,
                                in_=rd.unsqueeze(1).to_broadcast([1, 64, 1024]))
                            nc.vector.tensor_mul(
                                oTt[(h % 2) * 64:(h % 2) * 64 + 64,
                                    lhf * 1024:(lhf + 1) * 1024],
                                po[0:HD, :], rdb)

                        pending.append(tail)

                    # group-0 projections up front (k first: its chains drain
                    # under the q projections; E(0,0) needs k complete)
                    QK_ORDER = [("k", 0), ("k", 1), ("q", 0), ("q", 1)]
                    for nm, half in QK_ORDER:
                        u = qk_start(nm, 0, half)
                        qk_finish(u)
                        flush_pending()
                    for g in range(4):
                        if g >= 1:
                            oT.append(qkTp.tile([128, L], bf16, tag=f"qT{g-1}",
                                                name=f"oT{g}"))
                        e_units = [(2 * g + hh, lhf)
                                   for hh in range(2) for lhf in range(2)]
                        for j, (h, lhf) in enumerate(e_units):
                            qk = (qk_start(QK_ORDER[j][0], g + 1, QK_ORDER[j][1])
                                  if g < 3 else None)
                            e_unit(h, lhf, oT[g], qk)
                        if g == 2:
                            # prefetch output-projection weights into dead x
                            # slots (same shape) while group 3 attention runs
                            wo = []
                            for kj in range(4):
                                t = xsp.tile([128, D], bf16, tag=f"xs{kj}",
                                             name=f"wo{kj}")
                                nc.sync.dma_start(
                                    out=t, in_=woT_d[kj * 128:(kj + 1) * 128, :])
                                wo.append(t)
                    flush_pending()

                # ---- F: partial output projection ----
                with tc.tile_pool(name="psF", bufs=1, space="PSUM") as psF:
                    for m16 in range(16):
                        pf = psF.tile([128, L], f32, tag="pf", bufs=2)
                        for kj in range(4):
                            for n in range(4):
                                nc.tensor.matmul(
                                    pf[:, n * JL:(n + 1) * JL],
                                    lhsT=wo[kj][:, m16 * 128:(m16 + 1) * 128],
                                    rhs=oT[kj][:, n * JL:(n + 1) * JL],
                                    start=(kj == 0), stop=(kj == 3))
                        for hh in range(2):
                            fb = etp.tile([128, 1024], bf16, tag="et", name="fb",
                                          bufs=3)
                            nc.vector.tensor_copy(fb, pf[:, hh * 1024:(hh + 1) * 1024])
                            nc.sync.dma_start(
                                out=out_d[m16 * 128:(m16 + 1) * 128,
                                          hh * 1024:(hh + 1) * 1024],
                                in_=fb)
    return nc


_NC_CACHE = None


def kernel(**inputs):
    global _NC_CACHE
    from concourse.bass_utils import run_bass_kernel_spmd

    in_maps = _host_prep(
        np.asarray(inputs["x"], np.float32), np.asarray(inputs["condition"], np.float32),
        np.asarray(inputs["rope"], np.float32), np.asarray(inputs["w_ada"], np.float32),
        np.asarray(inputs["w_qkv"], np.float32), np.asarray(inputs["w_out"], np.float32),
        np.asarray(inputs["qk_w"], np.float32))
    if _NC_CACHE is None:
        _NC_CACHE = _build_nc()
        if not _NC_CACHE.is_finalized():
            _NC_CACHE.finalize()
    res = run_bass_kernel_spmd(_NC_CACHE, in_maps, list(range(NCORES)))
    out = np.zeros((B, L, D), np.float32)
    for b in range(B):
        acc = np.zeros((D, L), np.float32)
        for g in range(4):
            acc += res.results[b * 4 + g]["out"].astype(np.float32)
        out[b] = acc.T
    return out


# revision 3
# speedup vs baseline: 1.0050x; 1.0050x over previous
"""Trainium2 Bass kernel: fused AdaRMSNorm + QK-RMSNorm/RoPE attention.

Sharding: 8 cores = 2 batch x 4 head-groups (8 heads each).
vs v2: per-head RMS stats via gpsimd partition_all_reduce (no PE stat
matmuls, no PSUM pressure), attention interleaved with the q/k projection
per head-group, v computed first, shared PSUM tag for projection/score
matmuls, direct PSUM->DRAM output DMA.
"""

import numpy as np
from ml_dtypes import bfloat16

B, L, D, HD, DC = 2, 2048, 2048, 64, 2048
NH = D // HD
EPS = float(np.finfo(np.float32).eps)
NCORES = 8
JL = 512
NHL = 8


# ---------------------------------------------------------------- host prep
def _host_prep(x, condition, rope, w_ada, w_qkv, w_out, qk_w):
    hd_idx = np.arange(HD)
    sign = np.where(hd_idx % 2 == 0, -1.0, 1.0).astype(np.float32)
    partner = hd_idx ^ 1
    rope0, rope1 = rope[0].T, rope[1].T                      # [64, L]
    ropeA = (rope0 * qk_w[:, None]).astype(np.float32)
    ropeB = (rope1 * (sign * qk_w[partner])[:, None]).astype(np.float32)
    scl = np.float32(1.0 / np.sqrt(HD))
    ropeA2 = np.tile(ropeA, (2, 1))
    ropeB2 = np.tile(ropeB, (2, 1))
    bf = lambda a: np.ascontiguousarray(a.astype(bfloat16))
    ra_t = bf(ropeA2)
    rb_t = bf(ropeB2)

    in_maps = []
    for dev in range(NCORES):
        b, g = dev // 4, dev % 4
        ss = (w_ada @ condition[b]).astype(np.float32)
        shift, s1 = ss[:D], (1.0 + ss[D:]).astype(np.float32)
        Wq = w_qkv[g * JL:(g + 1) * JL]
        Wk = w_qkv[D + g * JL:D + (g + 1) * JL]
        Wv = w_qkv[2 * D + g * JL:2 * D + (g + 1) * JL]
        m = {
            "xT": bf(x[b].T),
            "wq": bf((Wq * s1[None, :]).T),
            "wk": bf((Wk * s1[None, :]).T),
            "wv": bf((Wv * s1[None, :]).T),
            "cqr": np.ascontiguousarray(
                (Wq @ shift).astype(bfloat16).reshape(1, 4, 128)),
            "ckr": np.ascontiguousarray(
                (Wk @ shift).astype(bfloat16).reshape(1, 4, 128)),
            "cqt": np.ascontiguousarray(
                (Wq @ shift).astype(np.float32).reshape(4, 128).T),
            "ckt": np.ascontiguousarray(
                (Wk @ shift).astype(np.float32).reshape(4, 128).T),
            "cv": (Wv @ shift).astype(bfloat16).reshape(1, JL),
            "ra": ra_t, "rb": rb_t,
            "woT": bf(w_out[:, g * JL:(g + 1) * JL].T),      # [512 j, 2048 d]
        }
        in_maps.append(m)
    return in_maps


# ---------------------------------------------------------------- bass build
def _build_nc():
    import concourse.bass as bass
    import concourse.mybir as mybir
    import concourse.tile as tile
    from concourse import bacc
    from concourse import bass_isa
    from concourse import library_config

    f32 = mybir.dt.float32
    f32r = mybir.dt.float32r
    bf16 = mybir.dt.bfloat16
    AF = mybir.ActivationFunctionType
    MUL = mybir.AluOpType.mult
    ADD = mybir.AluOpType.add

    nc = bacc.Bacc("TRN2", target_bir_lowering=False, debug=False, num_devices=8)
    xT_d = nc.dram_tensor("xT", [D, L], bf16, kind="ExternalInput")
    w_d = {nm: nc.dram_tensor(nm, [D, JL], bf16, kind="ExternalInput")
           for nm in ("wq", "wk", "wv")}
    cqt_d = nc.dram_tensor("cqr", [1, 4, 128], bf16, kind="ExternalInput")
    ckt_d = nc.dram_tensor("ckr", [1, 4, 128], bf16, kind="ExternalInput")
    cqs_d = nc.dram_tensor("cqt", [128, 4], f32, kind="ExternalInput")
    cks_d = nc.dram_tensor("ckt", [128, 4], f32, kind="ExternalInput")
    cv_d = nc.dram_tensor("cv", [1, JL], bf16, kind="ExternalInput")
    rope_d = {nm: nc.dram_tensor(nm, [128, L], bf16, kind="ExternalInput")
              for nm in ("ra", "rb")}
    woT_d = nc.dram_tensor("woT", [JL, D], bf16, kind="ExternalInput")
    out_d = nc.dram_tensor("out", [D, L], bf16, kind="ExternalOutput")

    def r(ap):
        return ap.bitcast(f32r)

    shuf = [i ^ 1 for i in range(32)]
    RADD = bass_isa.ReduceOp.add

    with tile.TileContext(nc) as tc, \
            nc.allow_low_precision(reason="bf16 kernel"):
        with (
            tc.tile_pool(name="consts", bufs=1) as consts,
            tc.tile_pool(name="qkT", bufs=1) as qkTp,
            tc.tile_pool(name="vsb", bufs=1) as vsbp,
            tc.tile_pool(name="dram", bufs=1, space="DRAM") as dpool,
        ):
            # preload the ln+exp activation table once; the fixpoint pass
            # then finds it resident on every path and inserts no reloads
            from concourse.hw_specs import get_activation_tables
            _tabs = list(get_activation_tables(nc.m.arch).keys())
            nc.scalar.add_instruction(mybir.InstLoadActFuncSet(
                name=nc.get_next_instruction_name(), ins=[], outs=[],
                act_func_set_id=_tabs.index("natural_log_exp_and_others")))
            ones_b = consts.tile([128, 1], bf16)
            nc.vector.memset(ones_b, 1.0)
            s2 = consts.tile([128, 2], bf16)
            nc.vector.memset(s2, 0.0)
            nc.vector.memset(s2[0:64, 0:1], 1.0)
            nc.vector.memset(s2[64:128, 1:2], 1.0)
            eps1 = consts.tile([1, 1], f32)
            nc.vector.memset(eps1, EPS)
            eps65 = consts.tile([65, 1], f32)
            nc.vector.memset(eps65, EPS)
            cq_t = consts.tile([1, 4, 128], bf16)
            nc.sync.dma_start(out=cq_t, in_=cqt_d[:, :, :])
            ck_t = consts.tile([1, 4, 128], bf16)
            nc.sync.dma_start(out=ck_t, in_=ckt_d[:, :, :])
            cq_s = consts.tile([128, 4], f32)
            nc.sync.dma_start(out=cq_s, in_=cqs_d[:, :])
            ck_s = consts.tile([128, 4], f32)
            nc.sync.dma_start(out=ck_s, in_=cks_d[:, :])
            cv_t = consts.tile([1, JL], bf16)
            nc.sync.dma_start(out=cv_t, in_=cv_d[:, :])
            cvb = consts.tile([128, JL], bf16)
            rr_row = consts.tile([1, L], bf16)
            rrb_sb = consts.tile([128, L], bf16)
            rr_col = consts.tile([128, 16], bf16)
            ones_row_b = consts.tile([1, 128], bf16)
            nc.vector.memset(ones_row_b, 1.0)

            # persistent bf16 tiles: q/k (roped+normalized) and v (+ones col)
            qkT = {"q": [], "k": []}
            for nm in ("q", "k"):
                for m in range(4):
                    qkT[nm].append(qkTp.tile([128, L], bf16, tag=f"{nm}T{m}",
                                             name=f"{nm}T{m}"))
            vsb = []
            for i in range(16):
                t = vsbp.tile([128, NHL, HD + 1], bf16, tag=f"v{i}", name=f"v{i}")
                nc.vector.memset(t[:, :, HD:HD + 1], 1.0)
                vsb.append(t)
            # oT tiles: oT[0] fresh, oT[g>=1] reuses the dead q-tile of group g-1
            oT = [qkTp.tile([128, L], bf16, tag="oT0", name="oT0")]

            with (
                tc.tile_pool(name="xs", bufs=1) as xsp,
                tc.tile_pool(name="wq", bufs=1) as wqp,
                tc.tile_pool(name="wvk", bufs=1) as wvkp,
                tc.tile_pool(name="ropes", bufs=1) as rp,
                tc.tile_pool(name="tmps", bufs=2) as tmp,
                tc.tile_pool(name="etp", bufs=1) as etp,
            ):
                # ---- A: stream x (+v weights), square, col sums-of-squares --
                xs, wtv = [], []
                with tc.tile_pool(name="psA1", bufs=1, space="PSUM") as psA1:
                    cvb_ps = psA1.tile([128, JL], f32, tag="cvb")
                    nc.tensor.matmul(cvb_ps, lhsT=ones_row_b, rhs=cv_t,
                                     start=True, stop=True)
                    nc.scalar.copy(cvb, cvb_ps)
                    ssq = psA1.tile([1, L], f32, tag="ssq")
                    for i in range(16):
                        t = xsp.tile([128, L], bf16, tag=f"xs{i}", name=f"xs{i}")
                        nc.sync.dma_start(out=t, in_=xT_d[i * 128:(i + 1) * 128, :])
                        xs.append(t)
                        tv = wvkp.tile([128, JL], bf16, tag=f"wk{i}", name=f"wv{i}")
                        nc.sync.dma_start(out=tv, in_=w_d["wv"][i * 128:(i + 1) * 128, :])
                        wtv.append(tv)
                        for hh in range(2):
                            sqa = tmp.tile([128, 1024], bf16, tag="sq1",
                                           name="sqA", bufs=2)
                            sl = slice(hh * 1024, (hh + 1) * 1024)
                            nc.vector.tensor_mul(sqa, t[:, sl], t[:, sl])
                            for c in range(2):
                                cg = hh * 2 + c
                                nc.tensor.matmul(ssq[:, cg * JL:(cg + 1) * JL],
                                                 lhsT=ones_b[:, 0:1],
                                                 rhs=sqa[:, c * JL:(c + 1) * JL],
                                                 start=(i == 0), stop=(i == 15))
                    nc.scalar.activation(ssq, ssq, AF.Ln, bias=eps1,
                                         scale=1.0 / D)
                    # rr_row = exp(0.5*ln(mean x^2 + eps)) = 1/rr; Ln+Exp share
                    # one activation table with the attention exps (no reloads)
                    nc.scalar.activation(rr_row, ssq, AF.Exp, scale=0.5)

                # ---- rr broadcasts + v projection ----
                with tc.tile_pool(name="psA2", bufs=1, space="PSUM") as psA2:
                    rr_scr = dpool.tile([16, 128], bf16, tag="rrscr")
                    nc.sync.dma_start(out=rr_scr,
                                      in_=rr_row.rearrange("a (m p) -> a m p", p=128))
                    inv_col = consts.tile([128, 16], bf16)
                    nc.sync.dma_start(out=inv_col,
                                      in_=rr_scr.rearrange("m p -> p m"))
                    nc.vector.reciprocal(rr_col, inv_col)
                    # prefetch q/k weights + rope tables (DMA slack here)
                    wtq = []
                    for k16 in range(16):
                        t = wqp.tile([128, JL], bf16, tag=f"wq{k16}",
                                     name=f"wq{k16}")
                        nc.sync.dma_start(
                            out=t, in_=w_d["wq"][k16 * 128:(k16 + 1) * 128, :])
                        wtq.append(t)
                    rope_t = {}
                    for nm in ("ra", "rb"):
                        rope_t[nm] = rp.tile([128, L], bf16, tag=nm, name=nm)
                        nc.sync.dma_start(out=rope_t[nm], in_=rope_d[nm][:, :])
                    cvb_r = cvb.rearrange("p (h d) -> p h d", h=NHL)
                    for mL in range(16):
                        pv = psA2.tile([128, JL], f32, tag="pv", bufs=3)
                        for k16 in range(16):
                            nc.tensor.matmul(
                                pv, lhsT=xs[k16][:, mL * 128:(mL + 1) * 128],
                                rhs=wtv[k16], start=(k16 == 0), stop=(k16 == 15))
                        nc.vector.scalar_tensor_tensor(
                            out=vsb[mL][:, :, 0:HD],
                            in0=pv.rearrange("p (h d) -> p h d", h=NHL),
                            scalar=rr_col[:, mL:mL + 1],
                            in1=cvb_r, op0=MUL, op1=ADD)
                    # rr broadcast for the q/k scale (needed from the first
                    # projection's m1 on; emitted after v so it cannot stall
                    # the PE stream on the adaRMS stats chain)
                    for c in range(4):
                        rrb_ps = psA2.tile([128, JL], f32, tag="pv", bufs=3)
                        nc.tensor.matmul(rrb_ps, lhsT=ones_row_b,
                                         rhs=rr_row[:, c * JL:(c + 1) * JL],
                                         start=True, stop=True)
                        nc.vector.reciprocal(rrb_sb[:, c * JL:(c + 1) * JL],
                                             rrb_ps)
                    # k weights into the wv slots (v matmuls hold them until done)
                    wtk = []
                    for k16 in range(16):
                        t = wvkp.tile([128, JL], bf16, tag=f"wk{k16}",
                                      name=f"wk{k16}")
                        nc.sync.dma_start(
                            out=t, in_=w_d["wk"][k16 * 128:(k16 + 1) * 128, :])
                        wtk.append(t)

                # ---- interleaved: per head-group, q/k proj+norm+rope and
                # attention, with projection matmuls spread through the
                # attention chunk stream so the scalar exp pipe never bunches.
                with tc.tile_pool(name="psCE", bufs=1, space="PSUM") as psCE:
                    pending = []  # deferred non-PE tails of prior units

                    def flush_pending():
                        while pending:
                            pending.pop(0)()

                    def qk_start(nm, m, half):
                        ps = psCE.tile([128, 1024], f32, tag="mm", bufs=1)
                        return {"nm": nm, "m": m, "half": half, "ps": ps,
                                "k16": 0, "mm_bias": m != 0}

                    def qk_mms(u, n):
                        ps, m, half = u["ps"], u["m"], u["half"]
                        wt = wtq if u["nm"] == "q" else wtk
                        for _ in range(n):
                            k16 = u["k16"]
                            if k16 >= 16:
                                return
                            u["k16"] += 1
                            for n2 in range(2):
                                nc.tensor.matmul(
                                    ps[:, n2 * JL:(n2 + 1) * JL],
                                    lhsT=wt[k16][:, m * 128:(m + 1) * 128],
                                    rhs=xs[k16][:, half * 1024 + n2 * JL:
                                                half * 1024 + (n2 + 1) * JL],
                                    start=(k16 == 0), stop=False)

                    def qk_finish(u):
                        nm, m, half, ps = u["nm"], u["m"], u["half"], u["ps"]
                        ra = rope_t["ra"]
                        rb = rope_t["rb"]
                        sl = slice(half * 1024, (half + 1) * 1024)
                        if u["mm_bias"]:
                            qk_mms(u, 16 - u["k16"])
                            c_t = cq_t if nm == "q" else ck_t
                            for n2 in range(2):
                                # bias row: + c[j]*(1/rr)[l]; the rr scale in
                                # m1 then yields rr*(Wx) + c exactly
                                nc.tensor.matmul(
                                    ps[:, n2 * JL:(n2 + 1) * JL],
                                    lhsT=c_t[0:1, m, :],
                                    rhs=rr_row[:, half * 1024 + n2 * JL:
                                               half * 1024 + (n2 + 1) * JL],
                                    start=False, stop=True)
                        else:
                            # group 0: close the accumulation on the last k16
                            # (no PE dependency on the adaRMS stats) and add
                            # the bias on the scalar engine instead
                            qk_mms(u, 15 - u["k16"])
                            wt = wtq if nm == "q" else wtk
                            for n2 in range(2):
                                nc.tensor.matmul(
                                    ps[:, n2 * JL:(n2 + 1) * JL],
                                    lhsT=wt[15][:, m * 128:(m + 1) * 128],
                                    rhs=xs[15][:, half * 1024 + n2 * JL:
                                               half * 1024 + (n2 + 1) * JL],
                                    start=False, stop=True)
                            u["k16"] = 16
                        m1 = tmp.tile([128, 1024], bf16, tag="m1", name="m1",
                                      bufs=2)
                        nc.vector.tensor_mul(m1, ps, rrb_sb[:, sl])
                        if not u["mm_bias"]:
                            c_s = cq_s if nm == "q" else ck_s
                            nc.scalar.activation(m1, m1, AF.Identity,
                                                 bias=c_s[:, m:m + 1])
                        sq = tmp.tile([128, 1024], bf16, tag="sq1", name="sq1",
                                      bufs=2)
                        nc.vector.tensor_mul(sq, m1, m1)
                        qs = tmp.tile([128, 1024], bf16, tag="qs", name="qs",
                                      bufs=2)
                        nc.vector.stream_shuffle(qs, m1, shuf)
                        # per-head sum of squares into PSUM rows 0 / 64
                        # (base partitions 0 and 64 are the HW-legal offsets)
                        ph = psCE.tile([128, 1024], f32, tag="pp", bufs=2)
                        for lc in range(2):
                            cs = slice(lc * JL, (lc + 1) * JL)
                            nc.tensor.matmul(ph[0:1, cs], lhsT=s2[:, 0:1],
                                             rhs=sq[:, cs], start=True, stop=True)
                            nc.tensor.matmul(ph[64:65, cs], lhsT=s2[:, 1:2],
                                             rhs=sq[:, cs], start=True, stop=True)
                        st = tmp.tile([128, 1024], bf16, tag="st", name="st",
                                      bufs=2)
                        nc.scalar.activation(ph[0:65, :], ph[0:65, :], AF.Ln,
                                             bias=eps65, scale=1.0 / HD)
                        nc.scalar.activation(st[0:65, :], ph[0:65, :], AF.Exp,
                                             scale=-0.5)
                        stb = tmp.tile([128, 1024], bf16, tag="stb",
                                       name="stb", bufs=2)
                        nc.sync.dma_start(
                            out=stb[0:64, :],
                            in_=st[0:1, :].unsqueeze(1).to_broadcast([1, 64, 1024]))
                        nc.sync.dma_start(
                            out=stb[64:128, :],
                            in_=st[64:65, :].unsqueeze(1).to_broadcast([1, 64, 1024]))

                        def tail():
                            nc.vector.tensor_mul(m1, m1, stb)        # qn
                            nc.vector.tensor_mul(qs, qs, stb)        # qn-shuf
                            nc.vector.tensor_mul(m1, m1, ra[:, sl])  # t1
                            nc.vector.tensor_mul(qs, qs, rb[:, sl])  # t2
                            nc.vector.tensor_add(qkT[nm][m][:, sl], m1, qs)

                        pending.append(tail)

                    def e_unit(h, lhf, oTt, qk=None):
                        qr_h = qkT["q"][h // 2][(h % 2) * 64:(h % 2) * 64 + 64, :]
                        kr_h = qkT["k"][h // 2][(h % 2) * 64:(h % 2) * 64 + 64, :]
                        ets = []

                        def s_chunk(l2c):
                            pS = psCE.tile([128, 1024], f32, tag="pp", bufs=2)
                            for n2 in range(2):
                                nc.tensor.matmul(
                                    pS[:, n2 * JL:(n2 + 1) * JL],
                                    lhsT=kr_h[:, l2c * 128:(l2c + 1) * 128],
                                    rhs=qr_h[:, lhf * 1024 + n2 * JL:
                                             lhf * 1024 + (n2 + 1) * JL],
                                    start=True, stop=True)
                            et = etp.tile([128, 1024], bf16, tag="et",
                                          name="et", bufs=4)
                            nc.scalar.activation(et, pS, AF.Exp,
                                                 scale=float(1.0 / np.sqrt(HD)))
                            ets.append(et)

                        s_chunk(0)
                        s_chunk(1)
                        flush_pending()
                        po = psCE.tile([HD + 1, 1024], f32, tag="po", bufs=1)

                        def po_chunk(l2c):
                            et = ets[l2c]
                            for n2 in range(2):
                                nc.tensor.matmul(
                                    po[:, n2 * JL:(n2 + 1) * JL],
                                    lhsT=vsb[l2c][:, h, :],
                                    rhs=et[:, n2 * JL:(n2 + 1) * JL],
                                    start=(l2c == 0), stop=(l2c == 15))

                        for l2c in range(2, 16):
                            s_chunk(l2c)
                            po_chunk(l2c - 2)
                            if qk is not None:
                                qk_mms(qk, 1)
                        po_chunk(14)
                        po_chunk(15)
                        if qk is not None:
                            qk_finish(qk)

                        def tail():
                            rd = tmp.tile([1, 1024], bf16, tag="rd", name="rd",
                                          bufs=1)
                            nc.vector.reciprocal(rd, po[HD:HD + 1, :])
                            rdb = tmp.tile([64, 1024], bf16, tag="rdb",
                                           name="rdb", bufs=1)
                            nc.sync.dma_start(
                                out=rdb,
                                in_=rd.unsqueeze(1).to_broadcast([1, 64, 1024]))
                            nc.vector.tensor_mul(
                                oTt[(h % 2) * 64:(h % 2) * 64 + 64,
                                    lhf * 1024:(lhf + 1) * 1024],
                                po[0:HD, :], rdb)

                        pending.append(tail)

                    # group-0 projections up front (k first: its chains drain
                    # under the q projections; E(0,0) needs k complete)
                    QK_ORDER = [("k", 0), ("k", 1), ("q", 0), ("q", 1)]
                    for nm, half in QK_ORDER:
                        u = qk_start(nm, 0, half)
                        qk_finish(u)
                        flush_pending()
                    for g in range(4):
                        if g >= 1:
                            oT.append(qkTp.tile([128, L], bf16, tag=f"qT{g-1}",
                                                name=f"oT{g}"))
                        e_units = [(2 * g + hh, lhf)
                                   for hh in range(2) for lhf in range(2)]
                        for j, (h, lhf) in enumerate(e_units):
                            qk = (qk_start(QK_ORDER[j][0], g + 1, QK_ORDER[j][1])
                                  if g < 3 else None)
                            e_unit(h, lhf, oT[g], qk)
                        if g == 2:
                            # prefetch output-projection weights into dead x
                            # slots (same shape) while group 3 attention runs
                            wo = []
                            for kj in range(4):
                                t = xsp.tile([128, D], bf16, tag=f"xs{kj}",
                                             name=f"wo{kj}")
                                nc.sync.dma_start(
                                    out=t, in_=woT_d[kj * 128:(kj + 1) * 128, :])
                                wo.append(t)
                    flush_pending()

                # ---- F: partial output projection ----
                with tc.tile_pool(name="psF", bufs=1, space="PSUM") as psF:
                    for m16 in range(16):
                        pf = psF.tile([128, L], f32, tag="pf", bufs=2)
                        for kj in range(4):
                            for n in range(4):
                                nc.tensor.matmul(
                                    pf[:, n * JL:(n + 1) * JL],
                                    lhsT=wo[kj][:, m16 * 128:(m16 + 1) * 128],
                                    rhs=oT[kj][:, n * JL:(n + 1) * JL],
                                    start=(kj == 0), stop=(kj == 3))
                        for hh in range(2):
                            fb = etp.tile([128, 1024], bf16, tag="et", name="fb",
                                          bufs=4)
                            nc.vector.tensor_copy(fb, pf[:, hh * 1024:(hh + 1) * 1024])
                            nc.sync.dma_start(
                                out=out_d[m16 * 128:(m16 + 1) * 128,
                                          hh * 1024:(hh + 1) * 1024],
                                in_=fb)
    return nc


_NC_CACHE = None


def kernel(**inputs):
    global _NC_CACHE
    from concourse.bass_utils import run_bass_kernel_spmd

    in_maps = _host_prep(
        np.asarray(inputs["x"], np.float32), np.asarray(inputs["condition"], np.float32),
        np.asarray(inputs["rope"], np.float32), np.asarray(inputs["w_ada"], np.float32),
        np.asarray(inputs["w_qkv"], np.float32), np.asarray(inputs["w_out"], np.float32),
        np.asarray(inputs["qk_w"], np.float32))
    if _NC_CACHE is None:
        _NC_CACHE = _build_nc()
        if not _NC_CACHE.is_finalized():
            _NC_CACHE.finalize()
    res = run_bass_kernel_spmd(_NC_CACHE, in_maps, list(range(NCORES)))
    out = np.zeros((B, L, D), np.float32)
    for b in range(B):
        acc = np.zeros((D, L), np.float32)
        for g in range(4):
            acc += res.results[b * 4 + g]["out"].astype(np.float32)
        out[b] = acc.T
    return out


# revision 4
# speedup vs baseline: 1.0102x; 1.0051x over previous
"""Trainium2 Bass kernel: fused AdaRMSNorm + QK-RMSNorm/RoPE attention.

Sharding: 8 cores = 2 batch x 4 head-groups (8 heads each).
vs v2: per-head RMS stats via gpsimd partition_all_reduce (no PE stat
matmuls, no PSUM pressure), attention interleaved with the q/k projection
per head-group, v computed first, shared PSUM tag for projection/score
matmuls, direct PSUM->DRAM output DMA.
"""

import numpy as np
from ml_dtypes import bfloat16

B, L, D, HD, DC = 2, 2048, 2048, 64, 2048
NH = D // HD
EPS = float(np.finfo(np.float32).eps)
NCORES = 8
JL = 512
NHL = 8


# ---------------------------------------------------------------- host prep
def _host_prep(x, condition, rope, w_ada, w_qkv, w_out, qk_w):
    hd_idx = np.arange(HD)
    sign = np.where(hd_idx % 2 == 0, -1.0, 1.0).astype(np.float32)
    partner = hd_idx ^ 1
    rope0, rope1 = rope[0].T, rope[1].T                      # [64, L]
    ropeA = (rope0 * qk_w[:, None]).astype(np.float32)
    ropeB = (rope1 * (sign * qk_w[partner])[:, None]).astype(np.float32)
    scl = np.float32(1.0 / np.sqrt(HD))
    ropeA2 = np.tile(ropeA, (2, 1))
    ropeB2 = np.tile(ropeB, (2, 1))
    bf = lambda a: np.ascontiguousarray(a.astype(bfloat16))
    ra_t = bf(ropeA2)
    rb_t = bf(ropeB2)

    in_maps = []
    for dev in range(NCORES):
        b, g = dev // 4, dev % 4
        ss = (w_ada @ condition[b]).astype(np.float32)
        shift, s1 = ss[:D], (1.0 + ss[D:]).astype(np.float32)
        Wq = w_qkv[g * JL:(g + 1) * JL]
        Wk = w_qkv[D + g * JL:D + (g + 1) * JL]
        Wv = w_qkv[2 * D + g * JL:2 * D + (g + 1) * JL]
        m = {
            "xT": bf(x[b].T),
            "wq": bf((Wq * s1[None, :]).T),
            "wk": bf((Wk * s1[None, :]).T),
            "wv": bf((Wv * s1[None, :]).T),
            "cqr": np.ascontiguousarray(
                (Wq @ shift).astype(bfloat16).reshape(1, 4, 128)),
            "ckr": np.ascontiguousarray(
                (Wk @ shift).astype(bfloat16).reshape(1, 4, 128)),
            "cqt": np.ascontiguousarray(
                (Wq @ shift).astype(np.float32).reshape(4, 128).T),
            "ckt": np.ascontiguousarray(
                (Wk @ shift).astype(np.float32).reshape(4, 128).T),
            "cv": (Wv @ shift).astype(bfloat16).reshape(1, JL),
            "ra": ra_t, "rb": rb_t,
            "woT": bf(w_out[:, g * JL:(g + 1) * JL].T),      # [512 j, 2048 d]
        }
        in_maps.append(m)
    return in_maps


# ---------------------------------------------------------------- bass build
def _build_nc():
    import concourse.bass as bass
    import concourse.mybir as mybir
    import concourse.tile as tile
    from concourse import bacc
    from concourse import bass_isa
    from concourse import library_config

    f32 = mybir.dt.float32
    f32r = mybir.dt.float32r
    bf16 = mybir.dt.bfloat16
    AF = mybir.ActivationFunctionType
    MUL = mybir.AluOpType.mult
    ADD = mybir.AluOpType.add

    nc = bacc.Bacc("TRN2", target_bir_lowering=False, debug=False, num_devices=8)
    xT_d = nc.dram_tensor("xT", [D, L], bf16, kind="ExternalInput")
    w_d = {nm: nc.dram_tensor(nm, [D, JL], bf16, kind="ExternalInput")
           for nm in ("wq", "wk", "wv")}
    cqt_d = nc.dram_tensor("cqr", [1, 4, 128], bf16, kind="ExternalInput")
    ckt_d = nc.dram_tensor("ckr", [1, 4, 128], bf16, kind="ExternalInput")
    cqs_d = nc.dram_tensor("cqt", [128, 4], f32, kind="ExternalInput")
    cks_d = nc.dram_tensor("ckt", [128, 4], f32, kind="ExternalInput")
    cv_d = nc.dram_tensor("cv", [1, JL], bf16, kind="ExternalInput")
    rope_d = {nm: nc.dram_tensor(nm, [128, L], bf16, kind="ExternalInput")
              for nm in ("ra", "rb")}
    woT_d = nc.dram_tensor("woT", [JL, D], bf16, kind="ExternalInput")
    out_d = nc.dram_tensor("out", [D, L], bf16, kind="ExternalOutput")

    def r(ap):
        return ap.bitcast(f32r)

    shuf = [i ^ 1 for i in range(32)]
    RADD = bass_isa.ReduceOp.add

    with tile.TileContext(nc) as tc, \
            nc.allow_low_precision(reason="bf16 kernel"):
        with (
            tc.tile_pool(name="consts", bufs=1) as consts,
            tc.tile_pool(name="qkT", bufs=1) as qkTp,
            tc.tile_pool(name="vsb", bufs=1) as vsbp,
            tc.tile_pool(name="dram", bufs=1, space="DRAM") as dpool,
        ):
            # preload the ln+exp activation table once; the fixpoint pass
            # then finds it resident on every path and inserts no reloads
            from concourse.hw_specs import get_activation_tables
            _tabs = list(get_activation_tables(nc.m.arch).keys())
            nc.scalar.add_instruction(mybir.InstLoadActFuncSet(
                name=nc.get_next_instruction_name(), ins=[], outs=[],
                act_func_set_id=_tabs.index("natural_log_exp_and_others")))
            ones_b = consts.tile([128, 1], bf16)
            nc.vector.memset(ones_b, 1.0)
            s2 = consts.tile([128, 2], bf16)
            nc.vector.memset(s2, 0.0)
            nc.vector.memset(s2[0:64, 0:1], 1.0)
            nc.vector.memset(s2[64:128, 1:2], 1.0)
            eps1 = consts.tile([1, 1], f32)
            nc.vector.memset(eps1, EPS)
            eps65 = consts.tile([65, 1], f32)
            nc.vector.memset(eps65, EPS)
            cq_t = consts.tile([1, 4, 128], bf16)
            nc.sync.dma_start(out=cq_t, in_=cqt_d[:, :, :])
            ck_t = consts.tile([1, 4, 128], bf16)
            nc.sync.dma_start(out=ck_t, in_=ckt_d[:, :, :])
            cq_s = consts.tile([128, 4], f32)
            nc.sync.dma_start(out=cq_s, in_=cqs_d[:, :])
            ck_s = consts.tile([128, 4], f32)
            nc.sync.dma_start(out=ck_s, in_=cks_d[:, :])
            cv_t = consts.tile([1, JL], bf16)
            nc.sync.dma_start(out=cv_t, in_=cv_d[:, :])
            cvb = consts.tile([128, JL], bf16)
            rr_row = consts.tile([1, L], bf16)
            rrb_sb = consts.tile([128, L], bf16)
            rr_col = consts.tile([128, 16], bf16)
            ones_row_b = consts.tile([1, 128], bf16)
            nc.vector.memset(ones_row_b, 1.0)

            # persistent bf16 tiles: q/k (roped+normalized) and v (+ones col)
            qkT = {"q": [], "k": []}
            for nm in ("q", "k"):
                for m in range(4):
                    qkT[nm].append(qkTp.tile([128, L], bf16, tag=f"{nm}T{m}",
                                             name=f"{nm}T{m}"))
            vsb = []
            for i in range(16):
                t = vsbp.tile([128, NHL, HD + 1], bf16, tag=f"v{i}", name=f"v{i}")
                nc.vector.memset(t[:, :, HD:HD + 1], 1.0)
                vsb.append(t)
            # oT tiles: oT[0] fresh, oT[g>=1] reuses the dead q-tile of group g-1
            oT = [qkTp.tile([128, L], bf16, tag="oT0", name="oT0")]

            with (
                tc.tile_pool(name="xs", bufs=1) as xsp,
                tc.tile_pool(name="wq", bufs=1) as wqp,
                tc.tile_pool(name="wvk", bufs=1) as wvkp,
                tc.tile_pool(name="ropes", bufs=1) as rp,
                tc.tile_pool(name="tmps", bufs=2) as tmp,
                tc.tile_pool(name="etp", bufs=1) as etp,
            ):
                # ---- A: stream x (+v weights), square, col sums-of-squares --
                xs, wtv = [], []
                with tc.tile_pool(name="psA1", bufs=1, space="PSUM") as psA1:
                    cvb_ps = psA1.tile([128, JL], f32, tag="cvb")
                    nc.tensor.matmul(cvb_ps, lhsT=ones_row_b, rhs=cv_t,
                                     start=True, stop=True)
                    nc.scalar.copy(cvb, cvb_ps)
                    ssq = psA1.tile([1, L], f32, tag="ssq")
                    for i in range(16):
                        t = xsp.tile([128, L], bf16, tag=f"xs{i}", name=f"xs{i}")
                        nc.sync.dma_start(out=t, in_=xT_d[i * 128:(i + 1) * 128, :])
                        xs.append(t)
                        tv = wvkp.tile([128, JL], bf16, tag=f"wk{i}", name=f"wv{i}")
                        nc.sync.dma_start(out=tv, in_=w_d["wv"][i * 128:(i + 1) * 128, :])
                        wtv.append(tv)
                        for hh in range(2):
                            sqa = tmp.tile([128, 1024], bf16, tag="sq1",
                                           name="sqA", bufs=2)
                            sl = slice(hh * 1024, (hh + 1) * 1024)
                            nc.vector.tensor_mul(sqa, t[:, sl], t[:, sl])
                            for c in range(2):
                                cg = hh * 2 + c
                                nc.tensor.matmul(ssq[:, cg * JL:(cg + 1) * JL],
                                                 lhsT=ones_b[:, 0:1],
                                                 rhs=sqa[:, c * JL:(c + 1) * JL],
                                                 start=(i == 0), stop=(i == 15))
                    nc.scalar.activation(ssq, ssq, AF.Ln, bias=eps1,
                                         scale=1.0 / D)
                    # rr_row = exp(0.5*ln(mean x^2 + eps)) = 1/rr; Ln+Exp share
                    # one activation table with the attention exps (no reloads)
                    nc.scalar.activation(rr_row, ssq, AF.Exp, scale=0.5)

                # ---- rr broadcasts + v projection ----
                with tc.tile_pool(name="psA2", bufs=1, space="PSUM") as psA2:
                    rr_scr = dpool.tile([16, 128], bf16, tag="rrscr")
                    nc.sync.dma_start(out=rr_scr,
                                      in_=rr_row.rearrange("a (m p) -> a m p", p=128))
                    inv_col = consts.tile([128, 16], bf16)
                    nc.sync.dma_start(out=inv_col,
                                      in_=rr_scr.rearrange("m p -> p m"))
                    nc.vector.reciprocal(rr_col, inv_col)
                    # prefetch q/k weights + rope tables (DMA slack here)
                    wtq = []
                    for k16 in range(16):
                        t = wqp.tile([128, JL], bf16, tag=f"wq{k16}",
                                     name=f"wq{k16}")
                        nc.sync.dma_start(
                            out=t, in_=w_d["wq"][k16 * 128:(k16 + 1) * 128, :])
                        wtq.append(t)
                    rope_t = {}
                    for nm in ("ra", "rb"):
                        rope_t[nm] = rp.tile([128, L], bf16, tag=nm, name=nm)
                        nc.sync.dma_start(out=rope_t[nm], in_=rope_d[nm][:, :])
                    cvb_r = cvb.rearrange("p (h d) -> p h d", h=NHL)
                    for mL in range(16):
                        pv = psA2.tile([128, JL], f32, tag="pv", bufs=3)
                        for k16 in range(16):
                            nc.tensor.matmul(
                                pv, lhsT=xs[k16][:, mL * 128:(mL + 1) * 128],
                                rhs=wtv[k16], start=(k16 == 0), stop=(k16 == 15))
                        nc.vector.scalar_tensor_tensor(
                            out=vsb[mL][:, :, 0:HD],
                            in0=pv.rearrange("p (h d) -> p h d", h=NHL),
                            scalar=rr_col[:, mL:mL + 1],
                            in1=cvb_r, op0=MUL, op1=ADD)
                    # rr broadcast for the q/k scale (needed from the first
                    # projection's m1 on; emitted after v so it cannot stall
                    # the PE stream on the adaRMS stats chain)
                    for c in range(4):
                        rrb_ps = psA2.tile([128, JL], f32, tag="pv", bufs=3)
                        nc.tensor.matmul(rrb_ps, lhsT=ones_row_b,
                                         rhs=rr_row[:, c * JL:(c + 1) * JL],
                                         start=True, stop=True)
                        nc.vector.reciprocal(rrb_sb[:, c * JL:(c + 1) * JL],
                                             rrb_ps)
                    # k weights into the wv slots (v matmuls hold them until done)
                    wtk = []
                    for k16 in range(16):
                        t = wvkp.tile([128, JL], bf16, tag=f"wk{k16}",
                                      name=f"wk{k16}")
                        nc.sync.dma_start(
                            out=t, in_=w_d["wk"][k16 * 128:(k16 + 1) * 128, :])
                        wtk.append(t)

                # ---- interleaved: per head-group, q/k proj+norm+rope and
                # attention, with projection matmuls spread through the
                # attention chunk stream so the scalar exp pipe never bunches.
                with tc.tile_pool(name="psCE", bufs=1, space="PSUM") as psCE:
                    pending = []  # deferred non-PE tails of prior units

                    def flush_pending():
                        while pending:
                            pending.pop(0)()

                    def qk_start(nm, m, half):
                        ps = psCE.tile([128, 1024], f32, tag="mm", bufs=1)
                        return {"nm": nm, "m": m, "half": half, "ps": ps,
                                "k16": 0, "mm_bias": m != 0}

                    def qk_mms(u, n):
                        ps, m, half = u["ps"], u["m"], u["half"]
                        wt = wtq if u["nm"] == "q" else wtk
                        for _ in range(n):
                            k16 = u["k16"]
                            if k16 >= 16:
                                return
                            u["k16"] += 1
                            for n2 in range(2):
                                nc.tensor.matmul(
                                    ps[:, n2 * JL:(n2 + 1) * JL],
                                    lhsT=wt[k16][:, m * 128:(m + 1) * 128],
                                    rhs=xs[k16][:, half * 1024 + n2 * JL:
                                                half * 1024 + (n2 + 1) * JL],
                                    start=(k16 == 0), stop=False)

                    def qk_finish(u):
                        nm, m, half, ps = u["nm"], u["m"], u["half"], u["ps"]
                        ra = rope_t["ra"]
                        rb = rope_t["rb"]
                        sl = slice(half * 1024, (half + 1) * 1024)
                        if u["mm_bias"]:
                            qk_mms(u, 16 - u["k16"])
                            c_t = cq_t if nm == "q" else ck_t
                            for n2 in range(2):
                                # bias row: + c[j]*(1/rr)[l]; the rr scale in
                                # m1 then yields rr*(Wx) + c exactly
                                nc.tensor.matmul(
                                    ps[:, n2 * JL:(n2 + 1) * JL],
                                    lhsT=c_t[0:1, m, :],
                                    rhs=rr_row[:, half * 1024 + n2 * JL:
                                               half * 1024 + (n2 + 1) * JL],
                                    start=False, stop=True)
                        else:
                            # group 0: close the accumulation on the last k16
                            # (no PE dependency on the adaRMS stats) and add
                            # the bias on the scalar engine instead
                            qk_mms(u, 15 - u["k16"])
                            wt = wtq if nm == "q" else wtk
                            for n2 in range(2):
                                nc.tensor.matmul(
                                    ps[:, n2 * JL:(n2 + 1) * JL],
                                    lhsT=wt[15][:, m * 128:(m + 1) * 128],
                                    rhs=xs[15][:, half * 1024 + n2 * JL:
                                               half * 1024 + (n2 + 1) * JL],
                                    start=False, stop=True)
                            u["k16"] = 16
                        m1 = tmp.tile([128, 1024], bf16, tag="m1", name="m1",
                                      bufs=2)
                        nc.vector.tensor_mul(m1, ps, rrb_sb[:, sl])
                        if not u["mm_bias"]:
                            c_s = cq_s if nm == "q" else ck_s
                            nc.scalar.activation(m1, m1, AF.Identity,
                                                 bias=c_s[:, m:m + 1])
                        sq = tmp.tile([128, 1024], bf16, tag="sq1", name="sq1",
                                      bufs=2)
                        nc.vector.tensor_mul(sq, m1, m1)
                        qs = tmp.tile([128, 1024], bf16, tag="qs", name="qs",
                                      bufs=2)
                        nc.vector.stream_shuffle(qs, m1, shuf)
                        # per-head sum of squares into PSUM rows 0 / 64
                        # (base partitions 0 and 64 are the HW-legal offsets)
                        ph = psCE.tile([128, 1024], f32, tag="pp", bufs=2)
                        for lc in range(2):
                            cs = slice(lc * JL, (lc + 1) * JL)
                            nc.tensor.matmul(ph[0:1, cs], lhsT=s2[:, 0:1],
                                             rhs=sq[:, cs], start=True, stop=True)
                            nc.tensor.matmul(ph[64:65, cs], lhsT=s2[:, 1:2],
                                             rhs=sq[:, cs], start=True, stop=True)
                        st = tmp.tile([128, 1024], bf16, tag="st", name="st",
                                      bufs=2)
                        nc.scalar.activation(ph[0:65, :], ph[0:65, :], AF.Ln,
                                             bias=eps65, scale=1.0 / HD)
                        nc.scalar.activation(st[0:65, :], ph[0:65, :], AF.Exp,
                                             scale=-0.5)
                        stb = tmp.tile([128, 1024], bf16, tag="stb",
                                       name="stb", bufs=2)
                        nc.gpsimd.dma_start(
                            out=stb[0:64, :],
                            in_=st[0:1, :].unsqueeze(1).to_broadcast([1, 64, 1024]))
                        nc.gpsimd.dma_start(
                            out=stb[64:128, :],
                            in_=st[64:65, :].unsqueeze(1).to_broadcast([1, 64, 1024]))

                        def tail():
                            nc.vector.tensor_mul(m1, m1, stb)        # qn
                            nc.vector.tensor_mul(qs, qs, stb)        # qn-shuf
                            nc.vector.tensor_mul(m1, m1, ra[:, sl])  # t1
                            nc.vector.tensor_mul(qs, qs, rb[:, sl])  # t2
                            nc.vector.tensor_add(qkT[nm][m][:, sl], m1, qs)

                        pending.append(tail)

                    def e_unit(h, lhf, oTt, qk=None):
                        qr_h = qkT["q"][h // 2][(h % 2) * 64:(h % 2) * 64 + 64, :]
                        kr_h = qkT["k"][h // 2][(h % 2) * 64:(h % 2) * 64 + 64, :]
                        ets = []

                        def s_chunk(l2c):
                            pS = psCE.tile([128, 1024], f32, tag="pp", bufs=2)
                            for n2 in range(2):
                                nc.tensor.matmul(
                                    pS[:, n2 * JL:(n2 + 1) * JL],
                                    lhsT=kr_h[:, l2c * 128:(l2c + 1) * 128],
                                    rhs=qr_h[:, lhf * 1024 + n2 * JL:
                                             lhf * 1024 + (n2 + 1) * JL],
                                    start=True, stop=True)
                            et = etp.tile([128, 1024], bf16, tag="et",
                                          name="et", bufs=4)
                            nc.scalar.activation(et, pS, AF.Exp,
                                                 scale=float(1.0 / np.sqrt(HD)))
                            ets.append(et)

                        s_chunk(0)
                        s_chunk(1)
                        flush_pending()
                        po = psCE.tile([HD + 1, 1024], f32, tag="po", bufs=1)

                        def po_chunk(l2c):
                            et = ets[l2c]
                            for n2 in range(2):
                                nc.tensor.matmul(
                                    po[:, n2 * JL:(n2 + 1) * JL],
                                    lhsT=vsb[l2c][:, h, :],
                                    rhs=et[:, n2 * JL:(n2 + 1) * JL],
                                    start=(l2c == 0), stop=(l2c == 15))

                        for l2c in range(2, 16):
                            s_chunk(l2c)
                            po_chunk(l2c - 2)
                            if qk is not None:
                                qk_mms(qk, 1)
                        po_chunk(14)
                        po_chunk(15)
                        if qk is not None:
                            qk_finish(qk)

                        def tail():
                            rd = tmp.tile([1, 1024], bf16, tag="rd", name="rd",
                                          bufs=1)
                            nc.vector.reciprocal(rd, po[HD:HD + 1, :])
                            rdb = tmp.tile([64, 1024], bf16, tag="rdb",
                                           name="rdb", bufs=1)
                            nc.gpsimd.dma_start(
                                out=rdb,
                                in_=rd.unsqueeze(1).to_broadcast([1, 64, 1024]))
                            nc.vector.tensor_mul(
                                oTt[(h % 2) * 64:(h % 2) * 64 + 64,
                                    lhf * 1024:(lhf + 1) * 1024],
                                po[0:HD, :], rdb)

                        pending.append(tail)

                    # group-0 projections up front (k first: its chains drain
                    # under the q projections; E(0,0) needs k complete)
                    QK_ORDER = [("k", 0), ("k", 1), ("q", 0), ("q", 1)]
                    for nm, half in QK_ORDER:
                        u = qk_start(nm, 0, half)
                        qk_finish(u)
                        flush_pending()
                    for g in range(4):
                        if g >= 1:
                            oT.append(qkTp.tile([128, L], bf16, tag=f"qT{g-1}",
                                                name=f"oT{g}"))
                        e_units = [(2 * g + hh, lhf)
                                   for hh in range(2) for lhf in range(2)]
                        for j, (h, lhf) in enumerate(e_units):
                            qk = (qk_start(QK_ORDER[j][0], g + 1, QK_ORDER[j][1])
                                  if g < 3 else None)
                            e_unit(h, lhf, oT[g], qk)
                        if g == 2:
                            # prefetch output-projection weights into dead x
                            # slots (same shape) while group 3 attention runs
                            wo = []
                            for kj in range(4):
                                t = xsp.tile([128, D], bf16, tag=f"xs{kj}",
                                             name=f"wo{kj}")
                                nc.sync.dma_start(
                                    out=t, in_=woT_d[kj * 128:(kj + 1) * 128, :])
                                wo.append(t)
                    flush_pending()

                # ---- F: partial output projection ----
                with tc.tile_pool(name="psF", bufs=1, space="PSUM") as psF:
                    for m16 in range(16):
                        pf = psF.tile([128, L], f32, tag="pf", bufs=2)
                        for kj in range(4):
                            for n in range(4):
                                nc.tensor.matmul(
                                    pf[:, n * JL:(n + 1) * JL],
                                    lhsT=wo[kj][:, m16 * 128:(m16 + 1) * 128],
                                    rhs=oT[kj][:, n * JL:(n + 1) * JL],
                                    start=(kj == 0), stop=(kj == 3))
                        for hh in range(2):
                            fb = etp.tile([128, 1024], bf16, tag="et", name="fb",
                                          bufs=4)
                            nc.vector.tensor_copy(fb, pf[:, hh * 1024:(hh + 1) * 1024])
                            nc.sync.dma_start(
                                out=out_d[m16 * 128:(m16 + 1) * 128,
                                          hh * 1024:(hh + 1) * 1024],
                                in_=fb)
    return nc


_NC_CACHE = None


def kernel(**inputs):
    global _NC_CACHE
    from concourse.bass_utils import run_bass_kernel_spmd

    in_maps = _host_prep(
        np.asarray(inputs["x"], np.float32), np.asarray(inputs["condition"], np.float32),
        np.asarray(inputs["rope"], np.float32), np.asarray(inputs["w_ada"], np.float32),
        np.asarray(inputs["w_qkv"], np.float32), np.asarray(inputs["w_out"], np.float32),
        np.asarray(inputs["qk_w"], np.float32))
    if _NC_CACHE is None:
        _NC_CACHE = _build_nc()
        if not _NC_CACHE.is_finalized():
            _NC_CACHE.finalize()
    res = run_bass_kernel_spmd(_NC_CACHE, in_maps, list(range(NCORES)))
    out = np.zeros((B, L, D), np.float32)
    for b in range(B):
        acc = np.zeros((D, L), np.float32)
        for g in range(4):
            acc += res.results[b * 4 + g]["out"].astype(np.float32)
        out[b] = acc.T
    return out
